# revision 4
# baseline (speedup 1.0000x reference)
"""ALCOVE cell Bass kernel for 8 TRN2 NeuronCores (data-parallel over batch).

B=32, T=16, N_RBF=1024, N_DIM=64, UNITS=64. 4 batches per core.

Layout: R=1024 on partitions as 8 chunks of 128; per-batch row data
(att, x, dx, g_att) on partition 0 as (1, B_LOC*64) rows (PE base-partition
rule); partition broadcasts via K=1 ones-matmul. Big elementwise work runs
on (128, B_LOC*NCHUNK*64) = (128, 2048) tiles in single instructions;
contractions over the free dim use TT-mult + tensor_reduce; contractions
over partitions use M=1 accumulating matmuls.
"""

import numpy as np

B, T, R, D, U = 32, 16, 1024, 64, 64
NCHUNK, P = 8, 128
EPS = 1e-6
N_CORES = 8
B_LOC = B // N_CORES  # 4

_cache = {}


def _patch_act_tables():
    """Make every activation resolve to natural_log_exp_and_others (it
    contains abs/ln/exp/relu/copy/identity/square) so the kernel needs a
    single ACT table load instead of thrashing between sets."""
    import concourse.bacc as bacc_mod
    from concourse.hw_specs import get_activation_tables as _gat

    if getattr(bacc_mod.get_activation_tables, "_alcove_patched", False):
        return

    def patched(arch):
        t = _gat(arch)
        keep = t["natural_log_exp_and_others"]
        out = {}
        for name, fns in t.items():
            out[name] = fns if name == "natural_log_exp_and_others" else (fns - keep)
        return out

    patched._alcove_patched = True
    bacc_mod.get_activation_tables = patched


def _build(rho, temperature, lr_att, lr_assoc, beta):
    import concourse.bass as bass
    import concourse.tile as tile
    from concourse import bacc, mybir

    _patch_act_tables()

    f32 = mybir.dt.float32
    bf16 = mybir.dt.bfloat16
    AF = mybir.ActivationFunctionType
    OP = mybir.AluOpType

    nc = bacc.Bacc("TRN2", target_bir_lowering=False, debug=False, num_devices=N_CORES)
    # packed bf16 input: [embedB (512) | zbcast (4096)]; f32 input: [oh (1024) | eye4 (4)]
    FINB = NCHUNK * D + T * B_LOC * D
    FIN = T * B_LOC * U + B_LOC
    bigb_in = nc.declare_dram_parameter("bigb", [P, FINB], bf16, isOutput=False)
    big_in = nc.declare_dram_parameter("big", [P, FIN], f32, isOutput=False)
    out_ext = nc.declare_dram_parameter("out", [B_LOC, T * U], f32, isOutput=True)

    with tile.TileContext(nc) as tc:
        with (
            tc.tile_pool(name="persist", bufs=1) as persist,
            tc.tile_pool(name="work", bufs=3) as work,
            tc.tile_pool(name="psum", bufs=1, space="PSUM") as psum,
            tc.tile_pool(name="psmall", bufs=2, space="PSUM") as psmall,
        ):
            # ---- persistent tiles (one DMA for all inputs) ----
            bigb = persist.tile([P, FINB], bf16)
            nc.gpsimd.dma_start(bigb[:], bigb_in[:])
            big = persist.tile([P, FIN], f32)
            nc.gpsimd.dma_start(big[:], big_in[:])
            embedB = bigb[:, 0 : NCHUNK * D]
            zb = bigb[:, NCHUNK * D :].rearrange("p (t f) -> p t f", t=T)
            # oh rows replicated on all partitions, layout (t, b, u)
            oh = big[0:B_LOC, 0 : T * B_LOC * U].rearrange("p (t b u) -> p t b u", t=T, b=B_LOC)
            eye4 = big[0:B_LOC, T * B_LOC * U :]  # (4, 4) identity
            eye_bc = eye4[:, :, None].broadcast_to([B_LOC, B_LOC, U])

            ones4 = persist.tile([B_LOC, P], bf16)
            nc.vector.memset(ones4[:], 1.0)
            consts = persist.tile([P, 3], f32)
            nc.vector.memset(consts[:, 0:1], 0.0)
            nc.vector.memset(consts[:, 1:2], 1.0)
            nc.vector.memset(consts[:, 2:3], EPS)
            czero, cone, ceps = consts[:, 0:1], consts[:, 1:2], consts[:, 2:3]

            attb_g = [persist.tile([P, B_LOC // 2, D], bf16, name=f"attb{g}") for g in range(2)]
            assoc_g = [persist.tile([P, B_LOC // 2, NCHUNK, U], bf16, name=f"assoc{g}") for g in range(2)]
            for g in range(2):
                nc.vector.memset(attb_g[g][:], 1.0 / D)
                nc.vector.memset(assoc_g[g][:], 0.0)

            probs_row = persist.tile([1, T, B_LOC, U], f32)


            BH = B_LOC // 2  # 2 batches per group
            # broadcast-view of embedB over group batches: (P, BH, NCHUNK, D)
            embed_bc = embedB.rearrange("p (c d) -> p c d", d=D)[:, None, :, :].broadcast_to([P, BH, NCHUNK, D])
            eye2_bc = eye4[0:BH, 0:BH][:, :, None].broadcast_to([BH, BH, U])
            ones2 = ones4[0:BH, :]

            for t in range(T):
                for g in range(2):
                    b0 = g * BH
                    attb = attb_g[g]
                    assoc = assoc_g[g]
                    # -------- diff^rho chain on (P, BH, NCHUNK, D)
                    zrep = zb[:, t, b0 * D : (b0 + BH) * D].rearrange("p (b d) -> p b d", d=D)[:, :, None, :].broadcast_to([P, BH, NCHUNK, D])
                    diff = work.tile([P, BH, NCHUNK, D], bf16, tag=f"diff{g}", name=f"diff{g}", bufs=3)
                    nc.gpsimd.tensor_tensor(diff[:], embed_bc, zrep, op=OP.subtract)
                    nc.scalar.activation(diff[:], diff[:], AF.Abs, bias=czero)
                    nc.scalar.activation(diff[:], diff[:], AF.Ln, bias=ceps)
                    dpow = work.tile([P, BH, NCHUNK, D], bf16, tag=f"dpow{g}", name=f"dpow{g}", bufs=3)
                    nc.scalar.activation(dpow[:], diff[:], AF.Exp, bias=czero, scale=rho)

                    # -------- q = sum_d att*dpow (att via stride-0 broadcast view)
                    qtmp = work.tile([P, BH, NCHUNK, D], bf16, tag=f"qtmp{g}", name=f"qtmp{g}", bufs=2)
                    nc.vector.tensor_tensor(qtmp[:], dpow[:],
                                            attb[:, :, None, :].broadcast_to([P, BH, NCHUNK, D]),
                                            op=OP.mult)
                    qall = work.tile([P, BH, NCHUNK], f32, tag=f"qall{g}", name=f"qall{g}")
                    nc.vector.tensor_reduce(qall[:], qtmp[:], axis=mybir.AxisListType.X, op=OP.add)

                    # -------- similarity acts
                    lnq = work.tile([P, BH, NCHUNK], f32, tag=f"lnq{g}", name=f"lnq{g}")
                    nc.scalar.activation(lnq[:], qall[:], AF.Ln, bias=ceps)
                    s_sim = work.tile([P, BH, NCHUNK], f32, tag=f"s_sim{g}", name=f"s_sim{g}")
                    nc.scalar.activation(s_sim[:], lnq[:], AF.Exp, bias=czero, scale=1.0 / rho)
                    nc.scalar.activation(s_sim[:], s_sim[:], AF.Exp, bias=czero, scale=-beta)
                    qp = work.tile([P, BH, NCHUNK], f32, tag=f"qp{g}", name=f"qp{g}")
                    nc.scalar.activation(qp[:], lnq[:], AF.Exp, bias=czero, scale=(1.0 - rho) / rho)
                    s_b16 = work.tile([P, BH, NCHUNK], bf16, tag=f"s_b16{g}", name=f"s_b16{g}")
                    nc.scalar.copy(s_b16[:], s_sim[:])

                    # -------- x: M=2 packed matmuls
                    x_ps = psmall.tile([BH, BH, U], f32, tag="x_ps", name="x_ps", bufs=2)
                    for c in range(NCHUNK):
                        nc.tensor.matmul(x_ps[:, :, :],
                                         s_b16[:, :, c],
                                         assoc[:, :, c, :],
                                         start=(c == 0), stop=(c == NCHUNK - 1))
                    # -------- teacher / dx on the cross tile
                    pp = work.tile([BH, BH, U], f32, tag=f"pp{g}", name=f"pp{g}")
                    nc.scalar.activation(pp[:], x_ps[:], AF.Relu, bias=cone[:BH, :])
                    mrow = work.tile([BH, BH, U], f32, tag=f"mrow{g}", name=f"mrow{g}")
                    nc.scalar.activation(mrow[:], x_ps[:], AF.Relu, bias=cone[:BH, :], scale=-1.0)
                    nc.vector.tensor_tensor(mrow[:], pp[:], mrow[:], op=OP.add)
                    nc.vector.tensor_tensor(mrow[:], mrow[:], oh[0:BH, t, b0 : b0 + BH, :], op=OP.mult)
                    dxf = work.tile([BH, BH, U], f32, tag=f"dxf{g}", name=f"dxf{g}")
                    nc.vector.tensor_tensor(dxf[:], pp[:], mrow[:], op=OP.subtract)
                    dxc = work.tile([BH, BH, U], bf16, tag=f"dxc{g}", name=f"dxc{g}")
                    nc.vector.tensor_tensor(dxc[:], dxf[:], eye2_bc, op=OP.mult)
                    xm = work.tile([BH, BH, U], bf16, tag=f"xm{g}", name=f"xm{g}")
                    nc.vector.tensor_tensor(xm[:], x_ps[:], eye2_bc, op=OP.mult)

                    # -------- dx broadcast to (P, BH, U)
                    dxb_ps = psum.tile([P, BH, U], f32, tag="dxb", name="dxb", bufs=2)
                    nc.tensor.matmul(dxb_ps[:, :, :].rearrange("p b d -> p (b d)"),
                                     ones2[:], dxc[:].rearrange("p b u -> p (b u)"),
                                     start=True, stop=True)
                    xrow_ps = psmall.tile([1, BH, U], f32, tag="rowps", name="rowps", bufs=1)
                    nc.tensor.matmul(xrow_ps[:, :, :].rearrange("p b u -> p (b u)"),
                                     ones2[:, 0:1], xm[:].rearrange("p b u -> p (b u)"),
                                     start=True, stop=True)

                    # -------- softmax in row layout
                    xr = work.tile([1, BH, U], f32, tag=f"xr{g}", name=f"xr{g}")
                    nc.scalar.copy(xr[:], xrow_ps[:])
                    mx4 = work.tile([1, BH], f32, tag=f"mx4{g}", name=f"mx4{g}")
                    nc.vector.tensor_reduce(mx4[:], xr[:], axis=mybir.AxisListType.X, op=OP.max)
                    mx_bc = mx4[:, :, None].broadcast_to([1, BH, U])
                    xs_t = work.tile([1, BH, U], f32, tag=f"xs_t{g}", name=f"xs_t{g}")
                    nc.gpsimd.tensor_tensor(xs_t[:], xr[:], mx_bc, op=OP.subtract)
                    nc.scalar.activation(probs_row[:, t, b0 : b0 + BH, :], xs_t[:], AF.Exp,
                                         bias=czero[:1, :], scale=temperature)
                    dxb_bc = dxb_ps[:, :, None, :].broadcast_to([P, BH, NCHUNK, U])

                    # -------- y = sum_u assoc*dx
                    ytmp = work.tile([P, BH, NCHUNK, U], bf16, tag=f"ytmp{g}", name=f"ytmp{g}", bufs=2)
                    nc.vector.tensor_tensor(ytmp[:], assoc[:], dxb_bc, op=OP.mult)
                    yall = work.tile([P, BH, NCHUNK], f32, tag=f"yall{g}", name=f"yall{g}")
                    nc.vector.tensor_reduce(yall[:], ytmp[:], axis=mybir.AxisListType.X, op=OP.add)

                    # -------- c = -(beta/rho) * s * qp * y
                    call = work.tile([P, BH, NCHUNK], f32, tag=f"call{g}", name=f"call{g}")
                    nc.vector.tensor_tensor(call[:], s_sim[:], qp[:], op=OP.mult)
                    nc.vector.scalar_tensor_tensor(call[:], yall[:], -beta / rho, call[:],
                                                   op0=OP.mult, op1=OP.mult)
                    call_b16 = work.tile([P, BH, NCHUNK], bf16, tag=f"call_b16{g}", name=f"call_b16{g}")
                    nc.scalar.copy(call_b16[:], call[:])

                    # -------- g_att + att update
                    gatt_ps = psmall.tile([BH, BH, D], f32, tag="gatt", name="gatt", bufs=2)
                    for c in range(NCHUNK):
                        nc.tensor.matmul(gatt_ps[:, :, :],
                                         call_b16[:, :, c],
                                         dpow[:, :, c, :],
                                         start=(c == 0), stop=(c == NCHUNK - 1))
                    gm = work.tile([BH, BH, D], bf16, tag=f"gm{g}", name=f"gm{g}")
                    nc.vector.tensor_tensor(gm[:], gatt_ps[:], eye2_bc, op=OP.mult)
                    grow_ps = psum.tile([P, BH, D], f32, tag="grow", name="grow", bufs=1)
                    nc.tensor.matmul(grow_ps[:, :, :].rearrange("p b d -> p (b d)"),
                                     ones2[:], gm[:].rearrange("p b d -> p (b d)"),
                                     start=True, stop=True)
                    nc.vector.scalar_tensor_tensor(attb[:], grow_ps[:], -lr_att, attb[:],
                                                   op0=OP.mult, op1=OP.add)
                    nc.vector.tensor_scalar_max(attb[:], attb[:], 0.0)

                    # -------- assoc update: upd = (dx * -lr) * s  (tiny slr, bcast views)
                    slr = work.tile([P, BH, NCHUNK], bf16, tag=f"slr{g}", name=f"slr{g}")
                    nc.vector.tensor_scalar_mul(slr[:], s_sim[:], -lr_assoc)
                    upd = work.tile([P, BH, NCHUNK, U], bf16, tag=f"upd{g}", name=f"upd{g}", bufs=2)
                    nc.vector.tensor_tensor(upd[:], dxb_bc,
                                            slr[:, :, :, None].broadcast_to([P, BH, NCHUNK, U]),
                                            op=OP.mult)
                    nc.vector.tensor_tensor(assoc[:], assoc[:], upd[:], op=OP.add)

            # -------- store: (1, T, B, U) row -> (B, T*U), one DMA per batch
            for b in range(B_LOC):
                nc.gpsimd.dma_start(out_ext[b : b + 1, :].rearrange("b (t u) -> b t u", t=T),
                                    probs_row[0:1, :, b, :])

    nc.compile()
    return nc


def _prep_in_maps(stimulus_set, label_idx, embed):
    embedB = embed.reshape(NCHUNK, P, D).transpose(1, 0, 2).reshape(P, NCHUNK * D)
    z = embed[stimulus_set]  # (B, T, D)
    onehot = np.zeros((B, T, U), dtype=np.float32)
    bi, ti = np.meshgrid(np.arange(B), np.arange(T), indexing="ij")
    onehot[bi, ti, label_idx] = 1.0
    in_maps = []
    for i in range(N_CORES):
        bs = slice(i * B_LOC, (i + 1) * B_LOC)
        zc = z[bs].transpose(1, 0, 2).reshape(1, T * B_LOC * D)
        zbcast = np.broadcast_to(zc, (P, T * B_LOC * D))
        ohrow = onehot[bs].transpose(1, 0, 2).reshape(1, T * B_LOC * U)
        ohfull = np.broadcast_to(ohrow, (P, T * B_LOC * U))
        eyefull = np.zeros((P, B_LOC), dtype=np.float32)
        eyefull[:B_LOC, :] = np.eye(B_LOC, dtype=np.float32)
        import ml_dtypes
        bigb = np.concatenate([embedB, zbcast], axis=1).astype(ml_dtypes.bfloat16)
        big = np.concatenate([ohfull, eyefull], axis=1).astype(np.float32)
        in_maps.append({"bigb": np.ascontiguousarray(bigb),
                        "big": np.ascontiguousarray(big)})
    return in_maps


def kernel(stimulus_set, label_idx, embed, rho, temperature, lr_attention, lr_association, beta):
    from concourse.bass_utils import run_bass_kernel_spmd

    stimulus_set = np.asarray(stimulus_set)
    label_idx = np.asarray(label_idx)
    embed = np.asarray(embed, dtype=np.float32)
    key = (float(rho), float(temperature), float(lr_attention),
           float(lr_association), float(beta))
    if key not in _cache:
        _cache[key] = _build(*key)
    nc = _cache[key]
    in_maps = _prep_in_maps(stimulus_set, label_idx, embed)
    res = run_bass_kernel_spmd(nc, in_maps, core_ids=list(range(N_CORES)))
    outs = [res.results[i]["out"].reshape(B_LOC, T, U) for i in range(N_CORES)]
    out = np.concatenate(outs, axis=0)
    return out / out.sum(axis=-1, keepdims=True)


def _install_ntff_hook():
    import sys, types, ctypes, contextlib
    if "antenv.axon_hooks" in sys.modules:
        return
    import antenv
    mod = types.ModuleType("antenv.axon_hooks")
    mod._hook = None
    def set_axon_ntff_profile_hook(h):
        mod._hook = h
    def get_axon_ntff_profile_hook():
        return mod._hook
    mod.set_axon_ntff_profile_hook = set_axon_ntff_profile_hook
    mod.get_axon_ntff_profile_hook = get_axon_ntff_profile_hook
    sys.modules["antenv.axon_hooks"] = mod
    antenv.axon_hooks = mod

    lib = ctypes.CDLL("/opt/axon/libaxon_pjrt.so")
    if not hasattr(lib, "axon_start_nrt_profile"):
        return
    lib.axon_start_nrt_profile.argtypes = [ctypes.POINTER(ctypes.c_int64), ctypes.c_size_t]
    lib.axon_start_nrt_profile.restype = ctypes.c_int64
    lib.axon_stop_nrt_profile.argtypes = [ctypes.c_char_p]
    lib.axon_stop_nrt_profile.restype = ctypes.c_int64

    @contextlib.contextmanager
    def _hook(output_dir, device_ids):
        import jax
        jax.devices()
        if device_ids:
            ids = (ctypes.c_int64 * len(device_ids))(*device_ids)
            rc = lib.axon_start_nrt_profile(ids, len(device_ids))
        else:
            rc = lib.axon_start_nrt_profile(None, 0)
        if rc != 0:
            raise RuntimeError(f"axon_start_nrt_profile rc={rc}")
        try:
            yield
        finally:
            n = lib.axon_stop_nrt_profile(str(output_dir).encode())
            print(f"profile: {n} file(s) written to {output_dir}")

    set_axon_ntff_profile_hook(_hook)


def kernel_traced(**inputs):
    """Like kernel() but runs with NTFF tracing; returns (out, exec_time_ns, tmpdir)."""
    import tempfile
    _install_ntff_hook()
    from concourse.bass_utils import run_bass_kernel_spmd

    key = (float(inputs["rho"]), float(inputs["temperature"]), float(inputs["lr_attention"]),
           float(inputs["lr_association"]), float(inputs["beta"]))
    if key not in _cache:
        _cache[key] = _build(*key)
    nc = _cache[key]
    in_maps = _prep_in_maps(np.asarray(inputs["stimulus_set"]), np.asarray(inputs["label_idx"]),
                            np.asarray(inputs["embed"], dtype=np.float32))
    tmpdir = tempfile.mkdtemp(prefix="alcove_trace_")
    res = run_bass_kernel_spmd(nc, in_maps, core_ids=list(range(N_CORES)), trace=True, tmpdir=tmpdir)
    outs = [res.results[i]["out"].reshape(B_LOC, T, U) for i in range(N_CORES)]
    out = np.concatenate(outs, axis=0)
    return out / out.sum(axis=-1, keepdims=True), res.exec_time_ns, tmpdir



# revision 15
# speedup vs baseline: 1.1638x; 1.1638x over previous
"""ALCOVE cell Bass kernel for 8 TRN2 NeuronCores (data-parallel over batch).

B=32, T=16, N_RBF=1024, N_DIM=64, UNITS=64. 4 batches per core.

Kernel-trick formulation: the output is only the softmax probs, and
assoc^t = -lr * sum_{tau<t} s^tau (x) dx^tau, so
  x^t = -lr * sum_tau (s^t . s^tau) dx^tau      (H-matmuls over R-chunks)
  y^t = assoc^t . dx^t = -lr * S_col @ g,  g_tau = dx^tau . dx^t
This removes all (R,U)-sized vector work (assoc tensor never exists).
R=1024 lives on partitions as 8 chunks of 128; per-step history tensors
S_col (P,BH,NCHUNK,T), DX_rowhist ((b,tau)=32 part, U) and DX_colhist
((b,u)=128 part, T) are appended each step and contracted with tiny
matmuls on the tensor engine.
"""

import numpy as np

B, T, R, D, U = 32, 16, 1024, 64, 64
NCHUNK, P = 8, 128
EPS = 1e-6
N_CORES = 8
B_LOC = B // N_CORES  # 4
BH = B_LOC // 2       # 2 batches per group

_cache = {}


def _patch_act_tables():
    """Make every activation resolve to natural_log_exp_and_others (it
    contains abs/ln/exp/relu/copy/identity/square) so the kernel needs a
    single ACT table load instead of thrashing between sets."""
    import concourse.bacc as bacc_mod
    from concourse.hw_specs import get_activation_tables as _gat

    if getattr(bacc_mod.get_activation_tables, "_alcove_patched", False):
        return

    def patched(arch):
        t = _gat(arch)
        keep = t["natural_log_exp_and_others"]
        out = {}
        for name, fns in t.items():
            out[name] = fns if name == "natural_log_exp_and_others" else (fns - keep)
        return out

    patched._alcove_patched = True
    bacc_mod.get_activation_tables = patched


def _build(rho, temperature, lr_att, lr_assoc, beta):
    import concourse.bass as bass
    import concourse.tile as tile
    from concourse import bacc, mybir

    _patch_act_tables()

    f32 = mybir.dt.float32
    bf16 = mybir.dt.bfloat16
    AF = mybir.ActivationFunctionType
    OP = mybir.AluOpType

    nc = bacc.Bacc("TRN2", target_bir_lowering=False, debug=False, num_devices=N_CORES)
    # packed bf16 input: [embedB (512) | zbcast (4096) | eye2T (2) | crow (3)]
    # f32 input: [oh 2 groups (2048, at partitions 0..1) | eye2 (2)]
    FINB = NCHUNK * D + T * B_LOC * D + 2 + 3
    FIN = 2 * T * U + 2
    bigb_in = nc.declare_dram_parameter("bigb", [P, FINB], bf16, isOutput=False)
    big_in = nc.declare_dram_parameter("big", [P, FIN], f32, isOutput=False)
    out_ext = nc.declare_dram_parameter("out", [B_LOC, T * U], f32, isOutput=True)

    with tile.TileContext(nc) as tc:
        with (
            tc.tile_pool(name="persist", bufs=1) as persist,
            tc.tile_pool(name="work", bufs=3) as work,
            tc.tile_pool(name="psum", bufs=1, space="PSUM") as psum,
            tc.tile_pool(name="psmall", bufs=2, space="PSUM") as psmall,
        ):
            # ---- persistent tiles (one DMA for all inputs) ----
            bigb = persist.tile([P, FINB], bf16)
            nc.gpsimd.dma_start(bigb[:], bigb_in[:])
            big = persist.tile([P, FIN], f32)
            nc.gpsimd.dma_start(big[:], big_in[:])
            embedB = bigb[:, 0 : NCHUNK * D]
            zb = bigb[:, NCHUNK * D : NCHUNK * D + T * B_LOC * D].rearrange(
                "p (t f) -> p t f", t=T)
            eye2T = bigb[0 : 2 * T, NCHUNK * D + T * B_LOC * D :][:, 0:2]  # (32,2) d(b=b')
            crow = bigb[0:BH, NCHUNK * D + T * B_LOC * D + 2 :]            # (2,3)
            # oh rows at partitions 0..BH-1, per group: (BH, T, U)
            oh_g = [big[0:BH, g * T * U : (g + 1) * T * U].rearrange(
                "p (t u) -> p t u", t=T) for g in range(2)]
            eye2 = big[0:BH, 2 * T * U :]  # (2,2) identity
            eye2_bc = eye2[:, :, None].broadcast_to([BH, BH, D])

            ones2 = persist.tile([BH, P], bf16)
            nc.vector.memset(ones2[:], 1.0)
            consts = persist.tile([P, 3], f32)
            nc.vector.memset(consts[:, 0:1], 0.0)
            nc.vector.memset(consts[:, 1:2], 1.0)
            nc.vector.memset(consts[:, 2:3], EPS)
            czero, cone, ceps = consts[:, 0:1], consts[:, 1:2], consts[:, 2:3]

            attb_g = [persist.tile([P, BH, D], bf16, name=f"attb{g}") for g in range(2)]
            S_col_g = [persist.tile([P, NCHUNK, BH, T], bf16, name=f"scol{g}") for g in range(2)]
            DXrow_g = [persist.tile([2 * T, U], bf16, name=f"dxrow{g}") for g in range(2)]
            DXcol_g = [persist.tile([P, T], bf16, name=f"dxcol{g}") for g in range(2)]
            gb_sb_g = [persist.tile([P, BH, T], bf16, name=f"gbsb{g}") for g in range(2)]
            gcross_g = [persist.tile([BH, BH, T], bf16, name=f"gcross{g}") for g in range(2)]
            probs_g = [persist.tile([BH, T, U], f32, name=f"probs{g}") for g in range(2)]
            for g in range(2):
                nc.vector.memset(attb_g[g][:], 1.0 / D)
                nc.vector.memset(S_col_g[g][:], 0.0)
                nc.vector.memset(DXrow_g[g][:], 0.0)
                nc.vector.memset(DXcol_g[g][:], 0.0)
                nc.vector.memset(gb_sb_g[g][:], 0.0)
                nc.vector.memset(gcross_g[g][:], 0.0)

            # broadcast-view of embedB over group batches: (P, BH, NCHUNK, D)
            embed_bc = embedB.rearrange("p (c d) -> p c d", d=D)[:, None, :, :].broadcast_to([P, BH, NCHUNK, D])
            kfold = beta * lr_assoc / rho

            for t in range(T):
                for g in range(2):
                    b0 = g * BH
                    attb = attb_g[g]
                    S_col = S_col_g[g]
                    DXrow = DXrow_g[g]
                    DXcol = DXcol_g[g]
                    gb_sb = gb_sb_g[g]
                    gcross = gcross_g[g]
                    # -------- diff^rho chain on (P, BH, NCHUNK, D)
                    zrep = zb[:, t, b0 * D : (b0 + BH) * D].rearrange("p (b d) -> p b d", d=D)[:, :, None, :].broadcast_to([P, BH, NCHUNK, D])
                    diff = work.tile([P, BH, NCHUNK, D], bf16, tag=f"diff{g}", name=f"diff{g}", bufs=3)
                    nc.gpsimd.tensor_tensor(diff[:], embed_bc, zrep, op=OP.subtract)
                    nc.scalar.activation(diff[:], diff[:], AF.Abs, bias=czero)
                    nc.scalar.activation(diff[:], diff[:], AF.Ln, bias=ceps)
                    dpow = work.tile([P, BH, NCHUNK, D], bf16, tag=f"dpow{g}", name=f"dpow{g}", bufs=3)
                    nc.scalar.activation(dpow[:], diff[:], AF.Exp, bias=czero, scale=rho)

                    # -------- q = sum_d att*dpow (att via stride-0 bcast view)
                    qtmp = work.tile([P, BH, NCHUNK, D], bf16, tag=f"qtmp{g}", name=f"qtmp{g}", bufs=2)
                    nc.vector.tensor_tensor(qtmp[:], dpow[:],
                                            attb[:, :, None, :].broadcast_to([P, BH, NCHUNK, D]),
                                            op=OP.mult)
                    qall = work.tile([P, BH, NCHUNK], f32, tag=f"qall{g}", name=f"qall{g}")
                    nc.vector.tensor_reduce(qall[:], qtmp[:], axis=mybir.AxisListType.X, op=OP.add)

                    # -------- similarity acts; s goes straight into S_col[.., t]
                    lnq = work.tile([P, BH, NCHUNK], f32, tag=f"lnq{g}", name=f"lnq{g}")
                    nc.scalar.activation(lnq[:], qall[:], AF.Ln, bias=ceps)
                    s_sim = work.tile([P, BH, NCHUNK], f32, tag=f"s_sim{g}", name=f"s_sim{g}")
                    nc.scalar.activation(s_sim[:], lnq[:], AF.Exp, bias=czero, scale=1.0 / rho)
                    nc.scalar.activation(s_sim[:], s_sim[:], AF.Exp, bias=czero, scale=-beta)
                    qp = work.tile([P, BH, NCHUNK], f32, tag=f"qp{g}", name=f"qp{g}")
                    nc.scalar.activation(qp[:], lnq[:], AF.Exp, bias=czero, scale=(1.0 - rho) / rho)
                    nc.scalar.copy(S_col[:, :, :, t], s_sim.rearrange("p b c -> p c b"))

                    # -------- H: h[(b,tau), b'] = sum_r S_col[r,b,tau] s_b'[r]
                    h_ps = psmall.tile([2 * T, BH], f32, tag="h_ps", name="h_ps", bufs=2)
                    for c in range(NCHUNK):
                        nc.tensor.matmul(h_ps[:, :],
                                         S_col[:, c, :, :],
                                         S_col[:, c, :, t],
                                         start=(c == 0), stop=(c == NCHUNK - 1))
                    # mask to diagonal batch & fold -lr_assoc
                    h_mask = work.tile([2 * T, BH], bf16, tag=f"hm{g}", name=f"hm{g}")
                    nc.vector.scalar_tensor_tensor(h_mask[:], h_ps[:], -lr_assoc, eye2T,
                                                   op0=OP.mult, op1=OP.mult)
                    # -------- x = h_mask^T @ DXrow : clean (BH, U)
                    x_ps = psmall.tile([BH, U], f32, tag="x_ps", name="x_ps", bufs=1)
                    nc.tensor.matmul(x_ps[:, :], h_mask[:], DXrow[:], start=True, stop=True)

                    # -------- teacher (col layout): dx = pp - oh*(pp+mrow)
                    pp = work.tile([BH, U], f32, tag=f"pp{g}", name=f"pp{g}")
                    nc.scalar.activation(pp[:], x_ps[:], AF.Relu, bias=cone[:BH, :])
                    mrow = work.tile([BH, U], f32, tag=f"mrow{g}", name=f"mrow{g}")
                    nc.scalar.activation(mrow[:], x_ps[:], AF.Relu, bias=cone[:BH, :], scale=-1.0)
                    nc.vector.tensor_tensor(mrow[:], pp[:], mrow[:], op=OP.add)
                    nc.vector.tensor_tensor(mrow[:], mrow[:], oh_g[g][:, t, :], op=OP.mult)
                    dxf = work.tile([BH, U], bf16, tag=f"dxf{g}", name=f"dxf{g}")
                    nc.vector.tensor_tensor(dxf[:], pp[:], mrow[:], op=OP.subtract)

                    # -------- softmax (col layout) -> probs
                    mx = work.tile([BH, 1], f32, tag=f"mx{g}", name=f"mx{g}")
                    nc.vector.tensor_reduce(mx[:], x_ps[:], axis=mybir.AxisListType.X, op=OP.max)
                    xs = work.tile([BH, U], f32, tag=f"xs{g}", name=f"xs{g}")
                    nc.vector.tensor_tensor(xs[:], x_ps[:], mx[:].broadcast_to([BH, U]), op=OP.subtract)
                    nc.scalar.activation(probs_g[g][:, t, :], xs[:], AF.Exp,
                                         bias=czero[:BH, :], scale=temperature)

                    # -------- dx transposes: psum cols [masked b0 | masked b1 | plain]
                    dxc = work.tile([BH, BH, U], bf16, tag=f"dxc{g}", name=f"dxc{g}")
                    nc.vector.tensor_tensor(dxc[:], dxf[:, None, :].broadcast_to([BH, BH, U]),
                                            eye2[:, :, None].broadcast_to([BH, BH, U]), op=OP.mult)
                    dxT_ps = psum.tile([P, 3], f32, tag="dxT", name="dxT", bufs=1)
                    nc.tensor.matmul(dxT_ps[:, :], dxc[:], crow[:], start=True, stop=True)
                    dxTm = work.tile([P, 2], bf16, tag=f"dxTm{g}", name=f"dxTm{g}")
                    nc.vector.tensor_copy(dxTm[:], dxT_ps[:, 0:2])
                    nc.scalar.copy(DXcol[:, t : t + 1], dxT_ps[:, 2:3])

                    if t > 0:
                        # -------- g_row[b, tau] = dx^tau_b . dx^t_b  (tau < t)
                        g_ps = psum.tile([BH, T], f32, tag="g_ps", name="g_ps", bufs=1)
                        nc.tensor.matmul(g_ps[:, 0:t], dxTm[:], DXcol[:, 0:t],
                                         start=True, stop=True)
                        # scale into masked cross rows (off-diag zeroed by eye)
                        nc.vector.scalar_tensor_tensor(gcross[:, :, 0:t],
                                                       g_ps[:, None, 0:t].broadcast_to([BH, BH, t]),
                                                       kfold,
                                                       eye2[:, :, None].broadcast_to([BH, BH, t]),
                                                       op0=OP.mult, op1=OP.mult)
                        # broadcast g rows to all partitions: sum_k gcross[k, (b,tau)]
                        gb_ps = psum.tile([P, BH, T], f32, tag="gb_ps", name="gb_ps", bufs=1)
                        nc.tensor.matmul(gb_ps[:, :, 0:t], ones2[:], gcross[:, :, 0:t],
                                         start=True, stop=True)
                        nc.scalar.copy(gb_sb[:, :, 0:t], gb_ps[:, :, 0:t])

                        # -------- y*k = sum_tau S_col * gb   (tiny free dim t)
                        ytmp = work.tile([P, NCHUNK, BH, T], bf16, tag=f"ytmp{g}", name=f"ytmp{g}", bufs=2)
                        nc.vector.tensor_tensor(ytmp[:, :, :, 0:t], S_col[:, :, :, 0:t],
                                                gb_sb[:, None, :, 0:t].broadcast_to([P, NCHUNK, BH, t]),
                                                op=OP.mult)
                        yall = work.tile([P, NCHUNK, BH], f32, tag=f"yall{g}", name=f"yall{g}")
                        nc.vector.tensor_reduce(yall[:], ytmp[:, :, :, 0:t],
                                                axis=mybir.AxisListType.X, op=OP.add)

                        # -------- c = s * qp * y*k   (k folds -beta/rho and -lr)
                        call = work.tile([P, BH, NCHUNK], f32, tag=f"call{g}", name=f"call{g}")
                        nc.vector.tensor_tensor(call[:], s_sim[:], qp[:], op=OP.mult)
                        call_b16 = work.tile([P, BH, NCHUNK], bf16, tag=f"call_b16{g}", name=f"call_b16{g}")
                        nc.vector.scalar_tensor_tensor(call_b16[:], yall.rearrange("p c b -> p b c"),
                                                       1.0, call[:], op0=OP.mult, op1=OP.mult)

                        # -------- g_att + att update
                        gatt_ps = psmall.tile([BH, BH, D], f32, tag="gatt", name="gatt", bufs=1)
                        for c in range(NCHUNK):
                            nc.tensor.matmul(gatt_ps[:, :, :],
                                             call_b16[:, :, c],
                                             dpow[:, :, c, :],
                                             start=(c == 0), stop=(c == NCHUNK - 1))
                        gm = work.tile([BH, BH, D], bf16, tag=f"gm{g}", name=f"gm{g}")
                        nc.vector.tensor_tensor(gm[:], gatt_ps[:], eye2_bc, op=OP.mult)
                        grow_ps = psum.tile([P, BH, D], f32, tag="grow", name="grow", bufs=1)
                        nc.tensor.matmul(grow_ps[:, :, :].rearrange("p b d -> p (b d)"),
                                         ones2[:], gm[:].rearrange("p b d -> p (b d)"),
                                         start=True, stop=True)
                        nc.vector.scalar_tensor_tensor(attb[:], grow_ps[:], -lr_att, attb[:],
                                                       op0=OP.mult, op1=OP.add)
                        nc.vector.tensor_scalar_max(attb[:], attb[:], 0.0)

                    # -------- append dx to row history (cross-partition: DMA)
                    for b in range(BH):
                        nc.gpsimd.dma_start(DXrow[b * T + t : b * T + t + 1, :],
                                            dxf[b : b + 1, :])

            # -------- store: per-batch DMA from (BH, T, U) col-layout probs
            for g in range(2):
                for i in range(BH):
                    b = g * BH + i
                    nc.gpsimd.dma_start(out_ext[b : b + 1, :].rearrange("b (t u) -> b t u", t=T),
                                        probs_g[g][i : i + 1, :, :])

    nc.compile()
    return nc


def _prep_in_maps(stimulus_set, label_idx, embed):
    import ml_dtypes
    embedB = embed.reshape(NCHUNK, P, D).transpose(1, 0, 2).reshape(P, NCHUNK * D)
    z = embed[stimulus_set]  # (B, T, D)
    onehot = np.zeros((B, T, U), dtype=np.float32)
    bi, ti = np.meshgrid(np.arange(B), np.arange(T), indexing="ij")
    onehot[bi, ti, label_idx] = 1.0
    # eye2T (32, 2): delta(p//T == j)
    eye2T = np.zeros((P, 2), dtype=np.float32)
    for p in range(2 * T):
        eye2T[p, p // T] = 1.0
    # crow (2, 3): row b = [b==0, b==1, 1]
    crow = np.zeros((P, 3), dtype=np.float32)
    crow[0, 0] = crow[1, 1] = crow[0, 2] = crow[1, 2] = 1.0
    eye2 = np.zeros((P, 2), dtype=np.float32)
    eye2[0, 0] = eye2[1, 1] = 1.0
    in_maps = []
    for i in range(N_CORES):
        bs = slice(i * B_LOC, (i + 1) * B_LOC)
        zc = z[bs].transpose(1, 0, 2).reshape(1, T * B_LOC * D)
        zbcast = np.broadcast_to(zc, (P, T * B_LOC * D))
        # oh at partitions 0..BH-1 per group: big[p, g*T*U + t*U + u] = onehot[g*BH+p]
        ohp = np.zeros((P, 2 * T * U), dtype=np.float32)
        for g in range(2):
            for p in range(BH):
                ohp[p, g * T * U : (g + 1) * T * U] = onehot[i * B_LOC + g * BH + p].reshape(-1)
        bigb = np.concatenate([embedB, zbcast, eye2T, crow], axis=1).astype(ml_dtypes.bfloat16)
        big = np.concatenate([ohp, eye2], axis=1).astype(np.float32)
        in_maps.append({"bigb": np.ascontiguousarray(bigb),
                        "big": np.ascontiguousarray(big)})
    return in_maps


def kernel(stimulus_set, label_idx, embed, rho, temperature, lr_attention, lr_association, beta):
    from concourse.bass_utils import run_bass_kernel_spmd

    stimulus_set = np.asarray(stimulus_set)
    label_idx = np.asarray(label_idx)
    embed = np.asarray(embed, dtype=np.float32)
    key = (float(rho), float(temperature), float(lr_attention),
           float(lr_association), float(beta))
    if key not in _cache:
        _cache[key] = _build(*key)
    nc = _cache[key]
    in_maps = _prep_in_maps(stimulus_set, label_idx, embed)
    res = run_bass_kernel_spmd(nc, in_maps, core_ids=list(range(N_CORES)))
    outs = [res.results[i]["out"].reshape(B_LOC, T, U) for i in range(N_CORES)]
    out = np.concatenate(outs, axis=0)
    return out / out.sum(axis=-1, keepdims=True)


def _install_ntff_hook():
    import sys, types, ctypes, contextlib
    if "antenv.axon_hooks" in sys.modules:
        return
    import antenv
    mod = types.ModuleType("antenv.axon_hooks")
    mod._hook = None
    def set_axon_ntff_profile_hook(h):
        mod._hook = h
    def get_axon_ntff_profile_hook():
        return mod._hook
    mod.set_axon_ntff_profile_hook = set_axon_ntff_profile_hook
    mod.get_axon_ntff_profile_hook = get_axon_ntff_profile_hook
    sys.modules["antenv.axon_hooks"] = mod
    antenv.axon_hooks = mod

    lib = ctypes.CDLL("/opt/axon/libaxon_pjrt.so")
    if not hasattr(lib, "axon_start_nrt_profile"):
        return
    lib.axon_start_nrt_profile.argtypes = [ctypes.POINTER(ctypes.c_int64), ctypes.c_size_t]
    lib.axon_start_nrt_profile.restype = ctypes.c_int64
    lib.axon_stop_nrt_profile.argtypes = [ctypes.c_char_p]
    lib.axon_stop_nrt_profile.restype = ctypes.c_int64

    @contextlib.contextmanager
    def _hook(output_dir, device_ids):
        import jax
        jax.devices()
        if device_ids:
            ids = (ctypes.c_int64 * len(device_ids))(*device_ids)
            rc = lib.axon_start_nrt_profile(ids, len(device_ids))
        else:
            rc = lib.axon_start_nrt_profile(None, 0)
        if rc != 0:
            raise RuntimeError(f"axon_start_nrt_profile rc={rc}")
        try:
            yield
        finally:
            n = lib.axon_stop_nrt_profile(str(output_dir).encode())
            print(f"profile: {n} file(s) written to {output_dir}")

    set_axon_ntff_profile_hook(_hook)


def kernel_traced(**inputs):
    """Like kernel() but runs with NTFF tracing; returns (out, exec_time_ns, tmpdir)."""
    import tempfile
    _install_ntff_hook()
    from concourse.bass_utils import run_bass_kernel_spmd

    key = (float(inputs["rho"]), float(inputs["temperature"]), float(inputs["lr_attention"]),
           float(inputs["lr_association"]), float(inputs["beta"]))
    if key not in _cache:
        _cache[key] = _build(*key)
    nc = _cache[key]
    in_maps = _prep_in_maps(np.asarray(inputs["stimulus_set"]), np.asarray(inputs["label_idx"]),
                            np.asarray(inputs["embed"], dtype=np.float32))
    tmpdir = tempfile.mkdtemp(prefix="alcove_trace_")
    res = run_bass_kernel_spmd(nc, in_maps, core_ids=list(range(N_CORES)), trace=True, tmpdir=tmpdir)
    outs = [res.results[i]["out"].reshape(B_LOC, T, U) for i in range(N_CORES)]
    out = np.concatenate(outs, axis=0)
    return out / out.sum(axis=-1, keepdims=True), res.exec_time_ns, tmpdir
